# revision 1
# baseline (speedup 1.0000x reference)
"""EdgeCrossingsLoss Trainium2 kernel (8-core SPMD, data-parallel over query faces).

Windowed kNN (spatial pruning): the host kd-sorts faces into 80 compact
tiles of 128 queries and, per tile, selects the W=552 candidates nearest
the tile's bounding box (point-to-box distance; deterministic hash shuffle
spreads spatial runs across top-8 chunks).  Per core, two device programs:

prog1 (10 tiles): PE computes -d2[q, c] for each tile's window via a 16-row
  bf16 hi/lo-split matmul (exact products, f32 PSUM), ACT evacuates
  PSUM->SBUF, DVE takes top-8 values + in-chunk indices over 4 chunks of
  160.  Inputs are packed into ONE dram tensor riding 2 DMAs, outputs
  stream out in 4 (each dma_start costs ~1.2us of shared HWDGE+SEQ time).  Window coverage is exact when the
  reported 16th distance is <= the window's point-to-box radius; rows
  violating that (or with a saturated chunk top-8) are recomputed on the
  host (~20% of rows, vectorized).

host: maps in-window indices to face ids, merges chunk-top-8s into the
  exact top-16 (value desc, index asc = the jax top_k tie-break), drops
  self (always rank 1), gathers 15 neighbor faces' f32 vertices (verts
  only: starts/dirs are rebuilt on device, halving DMA), folds
  probabilities into per-slot weights.

prog2: all 1280x15 3x3 line-line crossing tests, hit = num^2 <
  EPS^2*|b1 x b2|^2 (den=0/NaN falls out correctly).  Slot ranges are
  split across engines: DVE runs ~2/3 (squares offloaded to ACT with the
  EPS scale folded in, fused mask+reduce via scalar_tensor_tensor accum),
  Pool (gpsimd) runs a self-contained ~1/3 whose compare/mask/accum tail
  rides on DVE after its own emits - the ACT stream never couples the two
  ranges mid-chain.  All inputs arrive as one packed f32 tensor on 2 DMAs.
  (tensor_tensor_reduce and Pool scalar_tensor_tensor are rejected by this
  runtime; bf16 vertices shift near-miss pairs across the 1e-5 threshold
  and fail the 2e-2 gate - both found the hard way.)

Host sums the per-core partials and divides by num_faces.
"""
import os
import numpy as np
import ml_dtypes
from contextlib import ExitStack

import concourse.bass as bass
import concourse.tile as tile
import concourse.bacc as bacc
from concourse import mybir
from concourse.bass_utils import run_bass_kernel_spmd

F32 = mybir.dt.float32
BF16 = mybir.dt.bfloat16
U16 = mybir.dt.uint16

NCORES = 8
KNN = 16
EPS = 1e-5
FP = 10240            # padded query count
NR = FP // NCORES     # 1280 query rows per core
NT = NR // 128        # 10 tiles of 128 rows per core
NTILES = FP // 128    # 80 tiles total
W = 552               # candidate window per tile
NCH = 4               # top-8 chunks per window
CW = W // NCH         # 256-wide chunks
NC8 = NCH * 8         # 40 chunk-top-8 candidates per row
KMM = 16              # matmul contraction rows (bf16 hi/lo split)
MARGIN = 1e-3         # abs slack for device-vs-host value comparisons

ALU = mybir.AluOpType


def _build_prog1():
    nc = bacc.Bacc("TRN2", target_bir_lowering=False, debug=False,
                   num_devices=NCORES)
    # packed input: [lhsT NR | rhs NT*W] on the 16 contraction partitions
    pk1_in = nc.dram_tensor("pk1", [KMM, NR + NT * W], BF16,
                            kind="ExternalInput").ap()
    cv_out = nc.dram_tensor("cv", [NT, 128, NC8], F32, kind="ExternalOutput").ap()
    ci_out = nc.dram_tensor("ci", [NT, 128, NC8], U16, kind="ExternalOutput").ap()

    with tile.TileContext(nc) as tc, ExitStack() as ctx:
        const_pool = ctx.enter_context(tc.tile_pool(name="const", bufs=1))
        psum_pool = ctx.enter_context(tc.tile_pool(name="psum", bufs=3, space="PSUM"))
        negd2_pool = ctx.enter_context(tc.tile_pool(name="negd2", bufs=2))

        # batched DMA: each dma_start costs ~1.2us of shared HWDGE + SEQ
        # time.  sync carries lhsT + tile 0's window (unblocks compute),
        # scalar the remaining windows.
        pk1 = const_pool.tile([KMM, NR + NT * W], BF16)
        lhsT_sb = pk1[:, :NR]
        rhs_sb = pk1[:, NR:]
        nc.sync.dma_start(pk1[:, :NR + W], pk1_in[:, :NR + W])
        nc.scalar.dma_start(pk1[:, NR + W:], pk1_in[:, NR + W:])
        cv_all = const_pool.tile([128, NT, NC8], F32)
        ci_all = const_pool.tile([128, NT, NC8], U16)

        for t in range(NT):
            if t == NT - 1:
                # stream out everything but the last tile's slice so only
                # that slice remains on the post-compute tail
                nc.sync.dma_start(
                    cv_out[:NT - 1].rearrange("t p c -> p t c"),
                    cv_all[:, :NT - 1])
                nc.scalar.dma_start(
                    ci_out[:NT - 1].rearrange("t p c -> p t c"),
                    ci_all[:, :NT - 1])
            negd2 = negd2_pool.tile([128, W], F32, tag="negd2")
            for m in range(NCH):
                ps = psum_pool.tile([128, CW], F32, tag="ps")
                nc.tensor.matmul(
                    ps[:],
                    lhsT=lhsT_sb[:, t * 128:(t + 1) * 128],
                    rhs=rhs_sb[:, t * W + m * CW:t * W + (m + 1) * CW],
                    start=True, stop=True,
                )
                nc.scalar.copy(negd2[:, m * CW:(m + 1) * CW], ps[:])
                nc.vector.max(cv_all[:, t, m * 8:(m + 1) * 8],
                              negd2[:, m * CW:(m + 1) * CW])
                nc.vector.max_index(ci_all[:, t, m * 8:(m + 1) * 8],
                                    cv_all[:, t, m * 8:(m + 1) * 8],
                                    negd2[:, m * CW:(m + 1) * CW])
        nc.sync.dma_start(cv_out[NT - 1:].rearrange("t p c -> p t c"),
                          cv_all[:, NT - 1:])
        nc.scalar.dma_start(ci_out[NT - 1:].rearrange("t p c -> p t c"),
                            ci_all[:, NT - 1:])

    nc.compile()
    return nc


NS = 15               # neighbor slots per query (self is dropped on host)
TS2 = NT * NS         # 150 combined (tile, slot) units per core
# DVE is ~2x the throughput of Pool (gpsimd): DVE gets ~2/3 of the slots.
# The Pool range is one self-contained emit (its squares stay on Pool, the
# eps^2 scale folds into a final DVE STT) so the ACT stream never couples
# the two ranges mid-chain.
EMS_D = [60, 44]      # DVE emit sizes
PW = TS2 - sum(EMS_D)  # Pool range size (one emit)
EMS = EMS_D + [PW]
NEM = len(EMS)
EOFF = [sum(EMS[:i]) for i in range(NEM)]
POOL_E = NEM - 1


def _build_prog2():
    nc = bacc.Bacc("TRN2", target_bir_lowering=False, debug=False,
                   num_devices=NCORES)
    # f32 vertex coords, slot-minor layouts: posN[p, 3v*3c, ts]
    PKW = 9 * NT + 9 * TS2 + TS2
    pk_in = nc.dram_tensor("pk", [128, PKW], F32, kind="ExternalInput").ap()
    wcross_out = nc.dram_tensor("wcross", [128, NEM], F32, kind="ExternalOutput").ap()

    with tile.TileContext(nc) as tc, ExitStack() as ctx:
        pool = ctx.enter_context(tc.tile_pool(name="p", bufs=1))

        # one packed input tile; all views slice it.  Column layout:
        # [qpos 9*NT | posN chunks: e0, pool, e1 (9*EMS each) | vp TS2]
        pk = pool.tile([128, PKW], F32)
        off = [0, 9 * NT]
        for e in [0, POOL_E] + list(range(1, NEM - 1)):
            pass
        # chunk offsets in DMA/packing order e0, pool, e1
        PORD = [0, POOL_E] + list(range(1, NEM - 1))
        pko = {}
        o = 9 * NT
        for e in PORD:
            pko[e] = o
            o += 9 * EMS[e]
        VPOFF = o

        def pview(e):
            return pk[:, pko[e]:pko[e] + 9 * EMS[e]].rearrange(
                "p (r x) -> p r x", r=9)

        qposb = pk[:, :9 * NT].rearrange("p (r t) -> p r t", r=9)
        posN = [pview(e) for e in range(NEM)]
        vp = pk[:, VPOFF:VPOFF + TS2]
        # two DMAs: sync carries qpos + emit-0 verts (unblocks the first DVE
        # emit), scalar carries the rest
        SPLIT = pko[POOL_E]
        nc.sync.dma_start(pk[:, :SPLIT], pk_in[:, :SPLIT])
        nc.scalar.dma_start(pk[:, SPLIT:], pk_in[:, SPLIT:])

        # query-side edge geometry at tile granularity (tiny), then ACT
        # replicates it per neighbor slot.  edges: v0->v1, v0->v2, v1->v2
        qdir = pool.tile([128, 9, NT], F32)
        qsta = pool.tile([128, 9, NT], F32)
        nc.vector.tensor_tensor(qdir[:, 0:3], qposb[:, 3:6], qposb[:, 0:3],
                                ALU.subtract)
        nc.vector.tensor_tensor(qdir[:, 3:6], qposb[:, 6:9], qposb[:, 0:3],
                                ALU.subtract)
        nc.vector.tensor_tensor(qdir[:, 6:9], qposb[:, 6:9], qposb[:, 3:6],
                                ALU.subtract)
        nc.scalar.copy(qsta[:, 0:6].rearrange("p (e c) t -> p e c t", e=2),
                       qposb[:, 0:3].unsqueeze(1).broadcast_to([128, 2, 3, NT]))
        nc.scalar.copy(qsta[:, 6:9], qposb[:, 3:6])
        qdirR = pool.tile([128, 9, TS2], F32)
        qstaR = pool.tile([128, 9, TS2], F32)
        nc.scalar.copy(
            qdirR[:].rearrange("p r (t s) -> p r t s", t=NT),
            qdir[:].unsqueeze(3).broadcast_to([128, 9, NT, NS]))
        nc.scalar.copy(
            qstaR[:].rearrange("p r (t s) -> p r t s", t=NT),
            qsta[:].unsqueeze(3).broadcast_to([128, 9, NT, NS]))

        # all neighbor starts upfront on ACT (they only need the verts DMA)
        nsta = []
        for e in range(NEM):
            nx = EMS[e]
            ns = pool.tile([128, 9, nx], F32, name=f"nsta{e}")
            nc.scalar.copy(ns[:, 0:6].rearrange("p (e c) x -> p e c x", e=2),
                           posN[e][:, 0:3].unsqueeze(1)
                           .broadcast_to([128, 2, 3, nx]))
            nc.scalar.copy(ns[:, 6:9], posN[e][:, 3:6])
            nsta.append(ns)

        wacc = pool.tile([128, NEM], F32)
        pool_tail = []   # (cond-STT args) emitted on DVE after its own work

        def emit(e):
            """Edge tests for combined (tile, slot) range [x0, x1) on one
            engine.  SH = [p, query-edge, neighbor-edge, slot-range]."""
            x0, x1 = EOFF[e], EOFF[e] + EMS[e]
            nx = x1 - x0
            SH = [128, 3, 3, nx]
            xsl = slice(x0, x1)
            pfx = f"e{x0}"
            on_pool = e == POOL_E
            eng = nc.gpsimd if on_pool else nc.vector
            TT = eng.tensor_tensor

            ndir = pool.tile([128, 9, nx], F32, name=f"{pfx}_ndir")
            pN = posN[e]
            TT(ndir[:, 0:3], pN[:, 3:6], pN[:, 0:3], ALU.subtract)
            TT(ndir[:, 3:6], pN[:, 6:9], pN[:, 0:3], ALU.subtract)
            TT(ndir[:, 6:9], pN[:, 6:9], pN[:, 3:6], ALU.subtract)

            def uc(c):   # query edge dir comp c (varies qe axis)
                return qdirR[:, c:9:3, xsl].unsqueeze(2).broadcast_to(SH)

            def sc(c):   # query edge start comp c
                return qstaR[:, c:9:3, xsl].unsqueeze(2).broadcast_to(SH)

            def vc(c):   # neighbor edge dir comp c (varies ne axis)
                return ndir[:, c:9:3, :].unsqueeze(1).broadcast_to(SH)

            def tcp(c):  # neighbor edge start comp c
                return nsta[e][:, c:9:3, :].unsqueeze(1).broadcast_to(SH)

            m = [pool.tile(SH, F32, name=f"{pfx}_m{i}") for i in range(6)]
            dif = [pool.tile(SH, F32, name=f"{pfx}_d{i}") for i in range(3)]
            cr = [pool.tile(SH, F32, name=f"{pfx}_cr{i}") for i in range(3)]
            for i in range(3):  # cr_i = u_{i+1} * v_{i+2} - u_{i+2} * v_{i+1}
                a, b = (i + 1) % 3, (i + 2) % 3
                TT(m[2 * i][:], uc(a), vc(b), ALU.mult)
                TT(m[2 * i + 1][:], uc(b), vc(a), ALU.mult)
            for c in range(3):
                TT(dif[c][:], tcp(c), sc(c), ALU.subtract)
            for i in range(3):
                TT(cr[i][:], m[2 * i][:], m[2 * i + 1][:], ALU.subtract)

            num = pool.tile(SH, F32, name=f"{pfx}_num")
            t0 = pool.tile(SH, F32, name=f"{pfx}_t0")
            TT(num[:], dif[0][:], cr[0][:], ALU.mult)
            for c in (1, 2):
                TT(t0[:], dif[c][:], cr[c][:], ALU.mult)
                TT(num[:], num[:], t0[:], ALU.add)

            sq = [pool.tile(SH, F32, name=f"{pfx}_sq{i}") for i in range(3)]
            n2 = pool.tile(SH, F32, name=f"{pfx}_n2")
            den = pool.tile(SH, F32, name=f"{pfx}_den")
            if on_pool:
                # self-contained: squares on Pool (eps^2 folds into the
                # final DVE STT), compare+mask later on DVE
                for i in range(3):
                    TT(sq[i][:], cr[i][:], cr[i][:], ALU.mult)
                TT(n2[:], num[:], num[:], ALU.mult)
                TT(den[:], sq[0][:], sq[1][:], ALU.add)
                TT(den[:], den[:], sq[2][:], ALU.add)
                pool_tail.append((e, xsl, nx, den, n2, pfx))
                return
            # DVE range: eps^2*den2 = sum of (EPS*cr_i)^2 on ACT
            for i in range(3):
                nc.scalar.activation(sq[i][:], cr[i][:],
                                     mybir.ActivationFunctionType.Square,
                                     0.0, float(EPS))
            nc.scalar.activation(n2[:], num[:],
                                 mybir.ActivationFunctionType.Square, 0.0, 1.0)
            TT(den[:], sq[0][:], sq[1][:], ALU.add)
            TT(den[:], den[:], sq[2][:], ALU.add)
            cond = pool.tile(SH, F32, name=f"{pfx}_cond")
            TT(cond[:], den[:], n2[:], ALU.is_gt)     # num^2 < eps^2*|cr|^2
            scr = pool.tile(SH, F32, name=f"{pfx}_scr")
            nc.vector.scalar_tensor_tensor(
                scr[:].rearrange("p a b x -> p x (a b)"),
                cond[:].rearrange("p a b x -> p x (a b)"), 1.0,
                vp[:, xsl].unsqueeze(2).broadcast_to([128, nx, 9]),
                ALU.mult, ALU.mult, accum_out=wacc[:, e:e + 1])

        emit(POOL_E)                  # pool chain issues first, runs async
        for e in range(NEM - 1):
            emit(e)

        for e, xsl, nx, den, n2, pfx in pool_tail:
            cond = pool.tile([128, 3, 3, nx], F32, name=f"{pfx}_cond")
            nc.vector.scalar_tensor_tensor(
                cond[:], den[:], float(EPS * EPS), n2[:],
                ALU.mult, ALU.is_gt)
            scr = pool.tile([128, 3, 3, nx], F32, name=f"{pfx}_scr")
            nc.vector.scalar_tensor_tensor(
                scr[:].rearrange("p a b x -> p x (a b)"),
                cond[:].rearrange("p a b x -> p x (a b)"), 1.0,
                vp[:, xsl].unsqueeze(2).broadcast_to([128, nx, 9]),
                ALU.mult, ALU.mult, accum_out=wacc[:, e:e + 1])

        nc.sync.dma_start(wcross_out[:], wacc[:])

    nc.compile()
    return nc


_PROGS = {}


def _get_progs():
    if "p1" not in _PROGS:
        _PROGS["p1"] = _build_prog1()
        _PROGS["p2"] = _build_prog2()
    return _PROGS["p1"], _PROGS["p2"]


def _kd_order(pts, tile=128):
    """Order point ids so that each consecutive block of `tile` ids is
    spatially compact (recursive median splits along the widest axis).
    len(pts) must be a multiple of `tile`."""
    out = []

    def rec(idx):
        if len(idx) <= tile:
            out.append(idx)
            return
        p = pts[idx]
        ax = int(np.argmax(p.max(0) - p.min(0)))
        srt = idx[np.argsort(p[:, ax], kind="stable")]
        nl = (len(idx) // tile // 2) * tile
        rec(srt[:nl])
        rec(srt[nl:])

    rec(np.arange(len(pts)))
    return np.concatenate(out)


def _host_prep(vertices, faces, probabilities):
    V = np.ascontiguousarray(vertices, dtype=np.float32)
    Fc = np.ascontiguousarray(faces).astype(np.int64)
    P = np.ascontiguousarray(probabilities, dtype=np.float32)
    F = Fc.shape[0]

    pos = V[Fc]                                             # [F,3,3]
    bary = (pos[:, 0] + pos[:, 1] + pos[:, 2]) / np.float32(3.0)
    sq = (bary * bary).sum(-1, dtype=np.float32)

    # spatially-compact query tiles (pad queries parked far away)
    bary_pad = np.full((FP, 3), 1.0e9, np.float32)
    bary_pad[:F] = bary
    order = _kd_order(bary_pad)                             # [FP]
    is_real = order < F

    bf = ml_dtypes.bfloat16
    bh = bary.astype(bf).astype(np.float32)
    bl = (bary - bh).astype(bf).astype(np.float32)
    sqh = sq.astype(bf).astype(np.float32)
    sql = (sq - sqh).astype(bf).astype(np.float32)

    # per-tile candidate windows: the W candidates nearest the tile's box
    # (point-to-box distance).  R2 = w-th distance => coverage radius.
    win = np.empty((NTILES, W), np.int64)
    R2 = np.empty(NTILES, np.float64)
    b64 = bary.astype(np.float64)
    for t in range(NTILES):
        rq = order[t * 128:(t + 1) * 128]
        rq = rq[rq < F]
        if rq.size == 0:
            win[t] = np.arange(W)
            R2[t] = -1.0
            continue
        box_lo = b64[rq].min(0)
        box_hi = b64[rq].max(0)
        d = np.maximum(box_lo - b64, 0.0) + np.maximum(b64 - box_hi, 0.0)
        dbox2 = (d * d).sum(1)
        ids = np.argpartition(dbox2, W - 1)[:W]
        R2[t] = dbox2[ids].max()
        # deterministic hash shuffle so spatial NN runs spread across chunks
        h = (ids.astype(np.uint64) * np.uint64(2654435761)) & np.uint64(0xFFFFFFFF)
        win[t] = ids[np.argsort(h, kind="stable")]

    # rhs: per-tile window candidate columns [KMM, NTILES, W]
    rhs_rows = np.zeros((KMM, F), np.float32)
    rhs_rows[0:3] = (2.0 * bh).T
    rhs_rows[3:6] = (2.0 * bl).T
    rhs_rows[6:9] = (2.0 * bh).T
    rhs_rows[9:12] = (2.0 * bl).T
    rhs_rows[12] = -1.0
    rhs_rows[13] = -1.0
    rhs_rows[14] = -sqh
    rhs_rows[15] = -sql
    rhs_all = rhs_rows.astype(bf)                           # [KMM, F]
    rhs_t = rhs_all[:, win.reshape(-1)].reshape(KMM, NTILES, W)

    # lhsT: per-query columns in kd-sorted order; pad queries -> zeros
    lhsT = np.zeros((KMM, FP), np.float32)
    oc = np.where(is_real, order, 0)
    lhsT[0:3] = np.where(is_real, bh[oc].T, 0.0)
    lhsT[3:6] = np.where(is_real, bh[oc].T, 0.0)
    lhsT[6:9] = np.where(is_real, bl[oc].T, 0.0)
    lhsT[9:12] = np.where(is_real, bl[oc].T, 0.0)
    lhsT[12] = np.where(is_real, sqh[oc], 0.0)
    lhsT[13] = np.where(is_real, sql[oc], 0.0)
    lhsT[14] = np.where(is_real, 1.0, 0.0)
    lhsT[15] = np.where(is_real, 1.0, 0.0)
    lhsT_bf = lhsT.astype(bf)

    pos_bf = V[Fc]                                          # [F, 3, 3] f32

    in1 = []
    for c in range(NCORES):
        lo, hi = c * NR, (c + 1) * NR
        in1.append({"pk1": np.ascontiguousarray(np.concatenate(
            [lhsT_bf[:, lo:hi],
             rhs_t[:, c * NT:(c + 1) * NT].reshape(KMM, NT * W)], axis=1))})
    aux = dict(F=F, pos_bf=pos_bf, bary=bary, sq=sq, order=order,
               is_real=is_real, win=win, R2=R2, P=P)
    return in1, aux


def _exact_topk_rows(qids, aux):
    """Reference-style exact top-16 (f32 values, ties -> lowest face id)."""
    bary, sq, F = aux["bary"], aux["sq"], aux["F"]
    qb = bary[qids]                                         # [S,3]
    d2 = sq[qids][:, None] + sq[None, :] - 2.0 * (qb @ bary.T)
    part = np.argpartition(d2, KNN, axis=1)[:, :KNN]
    pv = np.take_along_axis(d2, part, axis=1)
    o = np.lexsort((part, pv), axis=1)
    return np.take_along_axis(part, o, axis=1)              # [S,16] face ids


def _host_merge(res1, aux):
    """Merge chunk-top-8s -> top-16 face ids per kd-sorted query row."""
    F, order, is_real = aux["F"], aux["order"], aux["is_real"]
    win, R2 = aux["win"], aux["R2"]

    vals = np.empty((FP, NC8), np.float32)
    lidx = np.empty((FP, NC8), np.int64)
    for c in range(NCORES):
        vals[c * NR:(c + 1) * NR] = \
            np.asarray(res1.results[c]["cv"]).reshape(NR, NC8)
        lidx[c * NR:(c + 1) * NR] = \
            np.asarray(res1.results[c]["ci"]).reshape(NR, NC8).astype(np.int64)
    tile_of = np.arange(FP) // 128
    wpos = (np.arange(NC8) // 8 * CW)[None, :] + lidx       # in-window position
    gidx = win[tile_of[:, None], wpos]                      # face ids [FP, NC8]

    part = np.argpartition(-vals, KNN, axis=1)[:, :KNN]
    pv = np.take_along_axis(vals, part, axis=1)
    pg = np.take_along_axis(gidx, part, axis=1)
    o = np.lexsort((pg, -pv), axis=1)
    nbr = np.take_along_axis(pg, o, axis=1)                 # [FP, 16]
    nv = np.take_along_axis(pv, o, axis=1)

    v16 = nv[:, KNN - 1]
    chunk8 = vals[:, 7::8]                                  # 8th value per chunk
    suspect = is_real & (
        (-v16 > R2[tile_of] - MARGIN)                       # coverage breach
        | (chunk8 >= (v16 - MARGIN)[:, None]).any(1))       # chunk truncation
    rows = np.nonzero(suspect)[0]
    _host_merge.last_suspects = int(rows.size)
    if rows.size:
        nbr[rows] = _exact_topk_rows(order[rows], aux)
    return nbr


def _run(vertices, faces, probabilities, trace=False, **kw):
    p1, p2 = _get_progs()
    in1, aux = _host_prep(vertices, faces, probabilities)
    res1 = run_bass_kernel_spmd(p1, in1, list(range(NCORES)), trace=trace, **kw)
    nbr = _host_merge(res1, aux)                            # [FP, 16] face ids
    F = aux["F"]
    order, is_real = aux["order"], aux["is_real"]

    # drop self (generically rank 1 of the top-16); stable-sort on the self
    # mask keeps the 15 non-self neighbors in merge order
    qid = np.where(is_real, order, -1)
    selfmask = nbr == qid[:, None]
    keep = np.argsort(selfmask, axis=1, kind="stable")[:, :NS]
    nbr15 = np.take_along_axis(nbr, keep, axis=1)           # [FP, 15]

    pos_bf = aux["pos_bf"]                                  # [F, 3, 3] f32
    posN = pos_bf[nbr15]                                    # [FP, 15, 3, 3]
    qpos = np.zeros((FP, 3, 3), pos_bf.dtype)
    qpos[is_real] = pos_bf[order[is_real]]
    qdirs = qpos[:, [1, 2, 2]] - qpos[:, [0, 0, 1]]         # [FP, 3e, 3c]
    qstas = qpos[:, [0, 0, 1]]
    probs_sorted = np.zeros(FP, np.float32)
    probs_sorted[is_real] = aux["P"][order[is_real]]
    vp = np.broadcast_to(probs_sorted[:, None], (FP, NS))

    PORD = [0, POOL_E] + list(range(1, NEM - 1))
    PKW = 9 * NT + 9 * TS2 + TS2
    in2 = []
    for c in range(NCORES):
        lo, hi = c * NR, (c + 1) * NR
        pN = posN[lo:hi].reshape(NT, 128, NS, 3, 3) \
            .transpose(1, 3, 4, 0, 2).reshape(128, 9, TS2)
        pkc = np.empty((128, PKW), np.float32)
        pkc[:, :9 * NT] = qpos[lo:hi].reshape(NT, 128, 3, 3) \
            .transpose(1, 2, 3, 0).reshape(128, 9 * NT)
        o = 9 * NT
        for e in PORD:
            x0 = EOFF[e]
            pkc[:, o:o + 9 * EMS[e]] = \
                pN[:, :, x0:x0 + EMS[e]].reshape(128, 9 * EMS[e])
            o += 9 * EMS[e]
        pkc[:, o:] = vp[lo:hi].reshape(NT, 128, NS) \
            .transpose(1, 0, 2).reshape(128, TS2)
        in2.append({"pk": pkc})
    res2 = run_bass_kernel_spmd(p2, in2, list(range(NCORES)), trace=trace, **kw)

    total = np.float64(0.0)
    for c in range(NCORES):
        total += np.asarray(res2.results[c]["wcross"], dtype=np.float64).sum()
    loss = np.float32(total / F)
    return loss, res1, res2, nbr


def run_device(vertices, faces, probabilities, trace=False, **kw):
    loss, res1, res2, _ = _run(vertices, faces, probabilities, trace=trace, **kw)
    return loss, (res1, res2)


def kernel(vertices, faces, probabilities):
    loss, *_ = _run(vertices, faces, probabilities)
    return np.array(loss, dtype=np.float32)



# revision 21
# speedup vs baseline: 1.1732x; 1.1732x over previous
"""EdgeCrossingsLoss Trainium2 kernel (8-core SPMD, data-parallel over query faces).

Windowed kNN (spatial pruning): the host kd-sorts faces into 80 compact
tiles of 128 queries and, per tile, selects the W=504 candidates nearest
the tile's bounding box (point-to-box distance; deterministic hash shuffle
spreads spatial runs across top-8 chunks).  Per core, two device programs:

prog1 (10 tiles): PE computes -d2[q, c] for each tile's window via a 16-row
  bf16 hi/lo-split matmul (exact products, f32 PSUM), DVE takes top-8
  values + in-chunk indices over 3 chunks of 168.  Tile 0's chunks are
  read directly from PSUM (skips the ACT hop on the critical startup
  path); later tiles go PSUM->SBUF on ACT so DVE scans SBUF (cheaper
  reads, ACT is otherwise idle).  cv (f32) and ci (u16, bitcast into the
  same f32 tile) ride ONE output tensor; tiles 0..8 stream out early.
  Window coverage is exact when the reported 16th distance is <= the
  window's point-to-box radius; rows violating that (or with a saturated
  chunk top-8) are recomputed on the host (vectorized).

host: maps in-window indices to face ids, merges chunk-top-8s into the
  exact top-16 (value desc, index asc = the jax top_k tie-break), drops
  self (always rank 1), and precomputes per-slot neighbor edge geometry:
  ndir (3 edge dirs) and dif12 (the 2x2 distinct start-start differences
  per component) - shipping these instead of raw vertices removes all
  subtract stages from the device hot loops (engines are the bottleneck,
  DMA is not).

prog2: all 1280x15 3x3 line-line crossing tests, hit = num^2 <
  EPS^2*|b1 x b2|^2 (den=0/NaN falls out correctly).  Four slot ranges:
  two on DVE, two on Pool (gpsimd), sized so both engines finish
  together.  Per range the engine computes only products: m (6), cr (3),
  num (5); ACT squares cr/num into bf16 (EPS scale folded in), expands
  dif12 -> dif27, and replicates query dirs per slot; DVE finishes each
  range with bf16 2x-mode den adds and a single fused
  (den-n2 > 0)*prob STT accumulation (also for the Pool ranges - Pool
  STT is rejected by this runtime).  Inputs arrive in 3 staged DMAs
  (pool range first - it starts the longest serial chain).  bf16 is only
  used AFTER the squares, where a 0.4% relative error cannot flip the
  hit test (f32 positions stay exact; bf16 positions were tried and fail
  the 2e-2 gate).

Host sums the per-core partials and divides by num_faces.
"""
import os
import numpy as np
import ml_dtypes
from contextlib import ExitStack

import concourse.bass as bass
import concourse.tile as tile
import concourse.bacc as bacc
from concourse import mybir
from concourse.bass_utils import run_bass_kernel_spmd

F32 = mybir.dt.float32
BF16 = mybir.dt.bfloat16
U16 = mybir.dt.uint16

NCORES = 8
KNN = 16
EPS = 1e-5
FP = 10240            # padded query count
NR = FP // NCORES     # 1280 query rows per core
NT = NR // 128        # 10 tiles of 128 rows per core
NTILES = FP // 128    # 80 tiles total
W = 504               # candidate window per tile
NCH = 3               # top-8 chunks per window
CW = W // NCH         # 168-wide chunks
NC8 = NCH * 8         # 24 chunk-top-8 candidates per row
KMM = 16              # matmul contraction rows (bf16 hi/lo split)
MARGIN = 1e-3         # abs slack for device-vs-host value comparisons
OC = NC8 + NC8 // 2   # 36 output cols per tile: 24 cv f32 + 24 ci u16

ALU = mybir.AluOpType


def _build_prog1():
    nc = bacc.Bacc("TRN2", target_bir_lowering=False, debug=False,
                   num_devices=NCORES)
    # packed input: [lhsT NR | rhs NT*W] on the 16 contraction partitions
    pk1_in = nc.dram_tensor("pk1", [KMM, NR + NT * W], BF16,
                            kind="ExternalInput").ap()
    cvci_out = nc.dram_tensor("cvci", [NT, 128, OC], F32,
                              kind="ExternalOutput").ap()

    with tile.TileContext(nc) as tc, ExitStack() as ctx:
        const_pool = ctx.enter_context(tc.tile_pool(name="const", bufs=1))
        psum_pool = ctx.enter_context(tc.tile_pool(name="psum", bufs=6, space="PSUM"))
        negd2_pool = ctx.enter_context(tc.tile_pool(name="negd2", bufs=6))

        # batched DMA: sync carries lhsT + tile 0's window (unblocks
        # compute), scalar the remaining windows.
        pk1 = const_pool.tile([KMM, NR + NT * W], BF16)
        lhsT_sb = pk1[:, :NR]
        rhs_sb = pk1[:, NR:]
        nc.sync.dma_start(pk1[:, :NR + W], pk1_in[:, :NR + W])
        nc.scalar.dma_start(pk1[:, NR + W:], pk1_in[:, NR + W:])
        cvci_all = const_pool.tile([128, NT, OC], F32)

        for t in range(NT):
            if t == NT - 1:
                # stream out everything but the last tile's slice so only
                # that slice remains on the post-compute tail
                nc.sync.dma_start(
                    cvci_out[:NT - 1].rearrange("t p c -> p t c"),
                    cvci_all[:, :NT - 1])
            civ_t = cvci_all[:, t, NC8:OC].bitcast(U16)     # [128, 24] u16
            if t == 0:
                # PSUM-direct top-8: skips the ACT copy on the startup path
                for m in range(NCH):
                    ps = psum_pool.tile([128, CW], F32, tag="ps")
                    nc.tensor.matmul(
                        ps[:],
                        lhsT=lhsT_sb[:, :128],
                        rhs=rhs_sb[:, m * CW:(m + 1) * CW],
                        start=True, stop=True,
                    )
                    nc.vector.max(cvci_all[:, 0, m * 8:(m + 1) * 8], ps[:])
                    nc.vector.max_index(civ_t[:, m * 8:(m + 1) * 8],
                                        cvci_all[:, 0, m * 8:(m + 1) * 8],
                                        ps[:])
                continue
            for m in range(NCH):
                ps = psum_pool.tile([128, CW], F32, tag="ps")
                nc.tensor.matmul(
                    ps[:],
                    lhsT=lhsT_sb[:, t * 128:(t + 1) * 128],
                    rhs=rhs_sb[:, t * W + m * CW:t * W + (m + 1) * CW],
                    start=True, stop=True,
                )
                negd2 = negd2_pool.tile([128, CW], F32, tag="negd2")
                nc.scalar.copy(negd2[:], ps[:])
                nc.vector.max(cvci_all[:, t, m * 8:(m + 1) * 8], negd2[:])
                nc.vector.max_index(civ_t[:, m * 8:(m + 1) * 8],
                                    cvci_all[:, t, m * 8:(m + 1) * 8],
                                    negd2[:])
        nc.sync.dma_start(cvci_out[NT - 1:].rearrange("t p c -> p t c"),
                          cvci_all[:, NT - 1:])

    nc.compile()
    return nc


NS = 15               # neighbor slots per query (self is dropped on host)
TS2 = NT * NS         # 150 combined (tile, slot) units per core
# Ranges in issue order; "p" = Pool (gpsimd), "e" = DVE.  Pool ranges are
# self-contained through den (products + squares + den adds on Pool,
# baseline-style) so only two f32 STTs per pool range ride DVE; DVE
# ranges get ACT squares (bf16, EPS folded) + a bf16 2x den/g tail and
# one fused (g>0)*prob STT.  DVE is ~2x Pool on the f32 product chain;
# Pool's self-contained rate is ~0.40us/slot vs DVE ~0.17 -> ~106/44
# split.  The first p/e pair rides lean early DMAs (dif12 expanded
# 12->27 on ACT); later ranges ship dif27 from the host.
RANGES = [("p", 29), ("e", 31), ("e", 31), ("e", 31), ("p", 28)]
NEM = len(RANGES)
assert sum(n for _, n in RANGES) == TS2
assert RANGES[0][0] == "p" and RANGES[1][0] == "e"
ROFF = [sum(n for _, n in RANGES[:i]) for i in range(NEM)]
NXS = [nx for _, nx in RANGES]
LATE_DVE = [r for r in range(2, NEM) if RANGES[r][0] == "e"]
LATE_POOL = [r for r in range(2, NEM) if RANGES[r][0] == "p"]
# tail groups: ROFF-contiguous runs of ranges whose den/g/STT merge into
# single wide DVE ops (amortizes the ~60ns per-op init); grouped by when
# their squares land
TGROUPS = [[0, 1, 2], [3], [4]]
assert [r for g in TGROUPS for r in g] == list(range(NEM))

# pk column layout = BLOCKS order; DMAS = (queue, n_blocks) consuming
# BLOCKS in order.  main = qdirR 9 + ndir 9 cols/slot; dif = 12 (early,
# ACT-expanded) or 27 (late, host-expanded) cols/slot.  Early blocks
# stay lean (transfers serialize on the shared DMA engines); only the
# first three DMAs ride the scalar queue so ACT's SEQ frees early.
BLOCKS = [("main", 0), ("main", 1), ("dif", 0), ("dif", 1),
          ("main", 2), ("main", 3), ("vp", -1), ("dif", 2),
          ("dif", 3), ("main", 4), ("dif", 4)]
DMAS = [("sync", 1), ("scalar", 1), ("scalar", 2), ("scalar", 1),
        ("sync", 2), ("sync", 1), ("sync", 1), ("sync", 1), ("sync", 1)]


def _block_cols(kind, r):
    if kind == "vp":
        return TS2
    if kind == "main":
        return 18 * NXS[r]
    return (12 if r < 2 else 27) * NXS[r]


def _build_prog2():
    nc = bacc.Bacc("TRN2", target_bir_lowering=False, debug=False,
                   num_devices=NCORES)
    pko = {}
    o = 0
    for kind, r in BLOCKS:
        pko[(kind, r)] = o
        o += _block_cols(kind, r)
    VPOFF = pko[("vp", -1)]
    PKW = o

    pk_in = nc.dram_tensor("pk", [128, PKW], F32, kind="ExternalInput").ap()
    wcross_out = nc.dram_tensor("wcross", [128, len(TGROUPS)], F32,
                                kind="ExternalOutput").ap()

    with tile.TileContext(nc) as tc, ExitStack() as ctx:
        pool = ctx.enter_context(tc.tile_pool(name="p", bufs=1))

        pk = pool.tile([128, PKW], F32)
        vp = pk[:, VPOFF:VPOFF + TS2]

        def qdirR_view(r):
            nx = NXS[r]
            o0 = pko[("main", r)]
            return pk[:, o0:o0 + 9 * nx].rearrange("p (e x) -> p e x", e=9)

        def ndir_view(r):
            nx = NXS[r]
            o0 = pko[("main", r)] + 9 * nx
            return pk[:, o0:o0 + 9 * nx].rearrange("p (e x) -> p e x", e=9)

        def dif12_view(r):  # [p, 2i, 2j, (3c x)]
            nx = NXS[r]
            o0 = pko[("dif", r)]
            return pk[:, o0:o0 + 12 * nx].rearrange(
                "p (i j cx) -> p i j cx", i=2, j=2)

        # input DMAs per the DMAS plan
        bi = 0
        for queue, nb in DMAS:
            k0, r0_ = BLOCKS[bi]
            o0 = pko[(k0, r0_)]
            o1 = o0 + sum(_block_cols(*BLOCKS[bi + j]) for j in range(nb))
            getattr(nc, queue).dma_start(pk[:, o0:o1], pk_in[:, o0:o1])
            bi += nb
        assert bi == len(BLOCKS)

        wacc = pool.tile([128, len(TGROUPS)], F32)
        nc.gpsimd.memset(wacc[:], 0.0)

        # dif27[r]: [p, 3qe, 3ne, (3c x)] — ACT-expanded for r0/r1,
        # direct pk views for the late ranges
        dif27 = {}

        def act_dif(r):
            nx = NXS[r]
            d27 = pool.tile([128, 3, 3, 3 * nx], F32, name=f"d27_{r}")
            s12 = dif12_view(r)
            nc.scalar.copy(d27[:, 0:2, 0:2],
                           s12[:, 0:1, 0:1].broadcast_to([128, 2, 2, 3 * nx]))
            nc.scalar.copy(d27[:, 0:2, 2:3],
                           s12[:, 0:1, 1:2].broadcast_to([128, 2, 1, 3 * nx]))
            nc.scalar.copy(d27[:, 2:3, 0:2],
                           s12[:, 1:2, 0:1].broadcast_to([128, 1, 2, 3 * nx]))
            nc.scalar.copy(d27[:, 2:3, 2:3], s12[:, 1:2, 1:2])
            dif27[r] = d27

        for r in range(2, NEM):
            nx = NXS[r]
            o0 = pko[("dif", r)]
            dif27[r] = pk[:, o0:o0 + 27 * nx].rearrange(
                "p (a b cx) -> p a b cx", a=3, b=3)

        chains = {}

        def emit_mcr(r):
            kind, nx = RANGES[r]
            SH = [128, 3, 3, nx]
            eng = nc.gpsimd if kind == "p" else nc.vector
            TT = eng.tensor_tensor
            pfx = f"r{r}"
            qd = qdirR_view(r)
            nd = ndir_view(r)

            def uc(c):   # query edge dir comp c (varies qe axis)
                return qd[:, c:9:3].unsqueeze(2).broadcast_to(SH)

            def vc(c):   # neighbor edge dir comp c (varies ne axis)
                return nd[:, c:9:3].unsqueeze(1).broadcast_to(SH)

            m = [pool.tile(SH, F32, name=f"{pfx}_m{i}") for i in range(6)]
            cr = [pool.tile(SH, F32, name=f"{pfx}_cr{i}") for i in range(3)]
            for i in range(3):  # cr_i = u_{i+1} * v_{i+2} - u_{i+2} * v_{i+1}
                a, b = (i + 1) % 3, (i + 2) % 3
                TT(m[2 * i][:], uc(a), vc(b), ALU.mult)
                TT(m[2 * i + 1][:], uc(b), vc(a), ALU.mult)
            for i in range(3):
                TT(cr[i][:], m[2 * i][:], m[2 * i + 1][:], ALU.subtract)
            chains[r] = cr

        def emit_num(r):
            kind, nx = RANGES[r]
            SH = [128, 3, 3, nx]
            eng = nc.gpsimd if kind == "p" else nc.vector
            TT = eng.tensor_tensor
            pfx = f"r{r}"
            cr = chains[r]

            def dc(c):   # start-start differences, expanded/shipped
                return dif27[r][:, :, :, c * nx:(c + 1) * nx]

            num = pool.tile(SH, F32, name=f"{pfx}_num")
            t0 = pool.tile(SH, F32, name=f"{pfx}_t0")
            TT(num[:], dc(0), cr[0][:], ALU.mult)
            for c in (1, 2):
                TT(t0[:], dc(c), cr[c][:], ALU.mult)
                TT(num[:], num[:], t0[:], ALU.add)
            chains[r] = (cr, num)

        # ---- ACT squares (bf16, EPS folded) into group-wide tiles ----
        gof = {}      # range -> (group idx, col offset, group tile set)
        gtiles = {}
        for gi, grp in enumerate(TGROUPS):
            g0 = ROFF[grp[0]]
            gnx = sum(NXS[r] for r in grp)
            GSH = [128, 3, 3, gnx]
            sqg = [pool.tile(GSH, BF16, name=f"g{gi}_sq{i}") for i in range(3)]
            n2g = pool.tile(GSH, BF16, name=f"g{gi}_n2")
            gtiles[gi] = (g0, gnx, sqg, n2g)
            for r in grp:
                gof[r] = (gi, ROFF[r] - g0)

        def act_squares(r):
            kind, nx = RANGES[r]
            cr, num = chains[r]
            gi, off = gof[r]
            _, _, sqg, n2g = gtiles[gi]
            for i in range(3):
                nc.scalar.activation(sqg[i][:, :, :, off:off + nx], cr[i][:],
                                     mybir.ActivationFunctionType.Square,
                                     0.0, float(EPS))
            nc.scalar.activation(n2g[:, :, :, off:off + nx], num[:],
                                 mybir.ActivationFunctionType.Square, 0.0, 1.0)

        # ---- DVE group tail: bf16 2x den adds + fused (den-n2>0)*vp ----
        def dve_tail_group(gi):
            g0, gnx, sqg, n2g = gtiles[gi]
            GSH = [128, 3, 3, gnx]
            den = pool.tile(GSH, BF16, name=f"g{gi}_den")
            g = pool.tile(GSH, BF16, name=f"g{gi}_g")
            nc.vector.tensor_tensor(den[:], sqg[0][:], sqg[1][:], ALU.add)
            nc.vector.tensor_tensor(den[:], den[:], sqg[2][:], ALU.add)
            nc.vector.tensor_tensor(g[:], den[:], n2g[:], ALU.subtract)
            scr = pool.tile(GSH, BF16, name=f"g{gi}_scr")
            nc.vector.scalar_tensor_tensor(
                scr[:].rearrange("p a b x -> p (a b) x"),
                g[:].rearrange("p a b x -> p (a b) x"), 0.0,
                vp[:, g0:g0 + gnx].unsqueeze(1).broadcast_to([128, 9, gnx]),
                ALU.is_gt, ALU.mult, accum_out=wacc[:, gi:gi + 1])

        # issue order: products first (pool p0 / dve e0 with early DMAs,
        # then the rest), ACT squares sequenced by when each range's cr
        # lands, group tails as soon as every member's squares land
        emit_mcr(0)               # pool p0
        emit_mcr(1)               # dve e0
        act_dif(0)
        emit_num(0)
        act_dif(1)
        emit_num(1)
        emit_mcr(2)
        emit_num(2)
        emit_mcr(3)
        emit_num(3)
        emit_mcr(4)               # pool p1
        emit_num(4)
        act_squares(1)
        act_squares(0)
        act_squares(2)
        dve_tail_group(0)
        act_squares(3)
        dve_tail_group(1)
        act_squares(4)
        dve_tail_group(2)

        nc.sync.dma_start(wcross_out[:], wacc[:])

    nc.compile()
    return nc


_PROGS = {}


def _get_progs():
    if "p1" not in _PROGS:
        _PROGS["p1"] = _build_prog1()
        _PROGS["p2"] = _build_prog2()
    return _PROGS["p1"], _PROGS["p2"]


def _kd_order(pts, tile=128):
    """Order point ids so that each consecutive block of `tile` ids is
    spatially compact (recursive median splits along the widest axis).
    len(pts) must be a multiple of `tile`."""
    out = []

    def rec(idx):
        if len(idx) <= tile:
            out.append(idx)
            return
        p = pts[idx]
        ax = int(np.argmax(p.max(0) - p.min(0)))
        srt = idx[np.argsort(p[:, ax], kind="stable")]
        nl = (len(idx) // tile // 2) * tile
        rec(srt[:nl])
        rec(srt[nl:])

    rec(np.arange(len(pts)))
    return np.concatenate(out)


def _host_prep(vertices, faces, probabilities):
    V = np.ascontiguousarray(vertices, dtype=np.float32)
    Fc = np.ascontiguousarray(faces).astype(np.int64)
    P = np.ascontiguousarray(probabilities, dtype=np.float32)
    F = Fc.shape[0]

    pos = V[Fc]                                             # [F,3,3]
    bary = (pos[:, 0] + pos[:, 1] + pos[:, 2]) / np.float32(3.0)
    sq = (bary * bary).sum(-1, dtype=np.float32)

    # spatially-compact query tiles (pad queries parked far away)
    bary_pad = np.full((FP, 3), 1.0e9, np.float32)
    bary_pad[:F] = bary
    order = _kd_order(bary_pad)                             # [FP]
    is_real = order < F

    bf = ml_dtypes.bfloat16
    bh = bary.astype(bf).astype(np.float32)
    bl = (bary - bh).astype(bf).astype(np.float32)
    sqh = sq.astype(bf).astype(np.float32)
    sql = (sq - sqh).astype(bf).astype(np.float32)

    # per-tile candidate windows: the W candidates nearest the tile's box
    # (point-to-box distance).  R2 = w-th distance => coverage radius.
    win = np.empty((NTILES, W), np.int64)
    R2 = np.empty(NTILES, np.float64)
    b64 = bary.astype(np.float64)
    for t in range(NTILES):
        rq = order[t * 128:(t + 1) * 128]
        rq = rq[rq < F]
        if rq.size == 0:
            win[t] = np.arange(W)
            R2[t] = -1.0
            continue
        box_lo = b64[rq].min(0)
        box_hi = b64[rq].max(0)
        d = np.maximum(box_lo - b64, 0.0) + np.maximum(b64 - box_hi, 0.0)
        dbox2 = (d * d).sum(1)
        ids = np.argpartition(dbox2, W - 1)[:W]
        R2[t] = dbox2[ids].max()
        # deterministic hash shuffle so spatial NN runs spread across chunks
        h = (ids.astype(np.uint64) * np.uint64(2654435761)) & np.uint64(0xFFFFFFFF)
        win[t] = ids[np.argsort(h, kind="stable")]

    # rhs: per-tile window candidate columns [KMM, NTILES, W]
    rhs_rows = np.zeros((KMM, F), np.float32)
    rhs_rows[0:3] = (2.0 * bh).T
    rhs_rows[3:6] = (2.0 * bl).T
    rhs_rows[6:9] = (2.0 * bh).T
    rhs_rows[9:12] = (2.0 * bl).T
    rhs_rows[12] = -1.0
    rhs_rows[13] = -1.0
    rhs_rows[14] = -sqh
    rhs_rows[15] = -sql
    rhs_all = rhs_rows.astype(bf)                           # [KMM, F]
    rhs_t = rhs_all[:, win.reshape(-1)].reshape(KMM, NTILES, W)

    # lhsT: per-query columns in kd-sorted order; pad queries -> zeros
    lhsT = np.zeros((KMM, FP), np.float32)
    oc = np.where(is_real, order, 0)
    lhsT[0:3] = np.where(is_real, bh[oc].T, 0.0)
    lhsT[3:6] = np.where(is_real, bh[oc].T, 0.0)
    lhsT[6:9] = np.where(is_real, bl[oc].T, 0.0)
    lhsT[9:12] = np.where(is_real, bl[oc].T, 0.0)
    lhsT[12] = np.where(is_real, sqh[oc], 0.0)
    lhsT[13] = np.where(is_real, sql[oc], 0.0)
    lhsT[14] = np.where(is_real, 1.0, 0.0)
    lhsT[15] = np.where(is_real, 1.0, 0.0)
    lhsT_bf = lhsT.astype(bf)

    pos_bf = V[Fc]                                          # [F, 3, 3] f32

    in1 = []
    for c in range(NCORES):
        lo, hi = c * NR, (c + 1) * NR
        in1.append({"pk1": np.ascontiguousarray(np.concatenate(
            [lhsT_bf[:, lo:hi],
             rhs_t[:, c * NT:(c + 1) * NT].reshape(KMM, NT * W)], axis=1))})
    aux = dict(F=F, pos_bf=pos_bf, bary=bary, sq=sq, order=order,
               is_real=is_real, win=win, R2=R2, P=P)
    return in1, aux


def _exact_topk_rows(qids, aux):
    """Reference-style exact top-16 (f32 values, ties -> lowest face id)."""
    bary, sq, F = aux["bary"], aux["sq"], aux["F"]
    qb = bary[qids]                                         # [S,3]
    d2 = sq[qids][:, None] + sq[None, :] - 2.0 * (qb @ bary.T)
    part = np.argpartition(d2, KNN, axis=1)[:, :KNN]
    pv = np.take_along_axis(d2, part, axis=1)
    o = np.lexsort((part, pv), axis=1)
    return np.take_along_axis(part, o, axis=1)              # [S,16] face ids


def _host_merge(res1, aux):
    """Merge chunk-top-8s -> top-16 face ids per kd-sorted query row."""
    F, order, is_real = aux["F"], aux["order"], aux["is_real"]
    win, R2 = aux["win"], aux["R2"]

    vals = np.empty((FP, NC8), np.float32)
    lidx = np.empty((FP, NC8), np.int64)
    for c in range(NCORES):
        cvci = np.asarray(res1.results[c]["cvci"])          # [NT, 128, OC] f32
        cv = cvci[:, :, :NC8]
        ci = cvci[:, :, NC8:].view(np.uint16)[:, :, :NC8]
        vals[c * NR:(c + 1) * NR] = cv.transpose(0, 1, 2).reshape(NR, NC8)
        lidx[c * NR:(c + 1) * NR] = ci.reshape(NR, NC8).astype(np.int64)
    tile_of = np.arange(FP) // 128
    wpos = (np.arange(NC8) // 8 * CW)[None, :] + lidx       # in-window position
    gidx = win[tile_of[:, None], wpos]                      # face ids [FP, NC8]

    part = np.argpartition(-vals, KNN, axis=1)[:, :KNN]
    pv = np.take_along_axis(vals, part, axis=1)
    pg = np.take_along_axis(gidx, part, axis=1)
    o = np.lexsort((pg, -pv), axis=1)
    nbr = np.take_along_axis(pg, o, axis=1)                 # [FP, 16]
    nv = np.take_along_axis(pv, o, axis=1)

    v16 = nv[:, KNN - 1]
    chunk8 = vals[:, 7::8]                                  # 8th value per chunk
    suspect = is_real & (
        (-v16 > R2[tile_of] - MARGIN)                       # coverage breach
        | (chunk8 >= (v16 - MARGIN)[:, None]).any(1))       # chunk truncation
    rows = np.nonzero(suspect)[0]
    _host_merge.last_suspects = int(rows.size)
    if rows.size:
        nbr[rows] = _exact_topk_rows(order[rows], aux)
    return nbr


def _run(vertices, faces, probabilities, trace=False, **kw):
    p1, p2 = _get_progs()
    in1, aux = _host_prep(vertices, faces, probabilities)
    res1 = run_bass_kernel_spmd(p1, in1, list(range(NCORES)), trace=trace, **kw)
    nbr = _host_merge(res1, aux)                            # [FP, 16] face ids
    F = aux["F"]
    order, is_real = aux["order"], aux["is_real"]

    # drop self (generically rank 1 of the top-16); stable-sort on the self
    # mask keeps the 15 non-self neighbors in merge order
    qid = np.where(is_real, order, -1)
    selfmask = nbr == qid[:, None]
    keep = np.argsort(selfmask, axis=1, kind="stable")[:, :NS]
    nbr15 = np.take_along_axis(nbr, keep, axis=1)           # [FP, 15]

    pos_bf = aux["pos_bf"]                                  # [F, 3, 3] f32
    posN = pos_bf[nbr15]                                    # [FP, 15, 3, 3]
    qpos = np.zeros((FP, 3, 3), pos_bf.dtype)
    qpos[is_real] = pos_bf[order[is_real]]
    qdirs = qpos[:, [1, 2, 2]] - qpos[:, [0, 0, 1]]         # [FP, 3e, 3c]
    # neighbor edge dirs + start-start differences
    ndirs = posN[:, :, [1, 2, 2]] - posN[:, :, [0, 0, 1]]   # [FP, 15, 3e, 3c]
    # dif12[q, s, i, j, c] = posN_start_j - qpos_start_i  (starts = v0, v1)
    nstarts = posN[:, :, [0, 1], :]                         # [FP, 15, 2j, 3c]
    qstarts = qpos[:, [0, 1], :]                            # [FP, 2i, 3c]
    dif12 = (nstarts[:, :, None, :, :]
             - qstarts[:, None, :, None, :])                # [FP, 15, 2i, 2j, 3c]
    SMAP = np.array([0, 0, 1])
    dif27 = dif12[:, :, SMAP[:, None], SMAP[None, :], :]    # [FP, 15, 3qe, 3ne, 3c]
    probs_sorted = np.zeros(FP, np.float32)
    probs_sorted[is_real] = aux["P"][order[is_real]]
    vp = np.broadcast_to(probs_sorted[:, None], (FP, NS))

    in2 = []
    for c in range(NCORES):
        lo, hi = c * NR, (c + 1) * NR
        # per-core slot-major views, slot u = t*NS + s
        qd_c = np.broadcast_to(
            qdirs[lo:hi].reshape(NT, 128, 1, 3, 3),
            (NT, 128, NS, 3, 3)) \
            .transpose(1, 3, 4, 0, 2).reshape(128, 9, TS2)
        nd_c = ndirs[lo:hi].reshape(NT, 128, NS, 3, 3) \
            .transpose(1, 3, 4, 0, 2).reshape(128, 9, TS2)
        df12_c = dif12[lo:hi].reshape(NT, 128, NS, 2, 2, 3) \
            .transpose(1, 3, 4, 5, 0, 2).reshape(128, 12, TS2)
        df27_c = dif27[lo:hi].reshape(NT, 128, NS, 3, 3, 3) \
            .transpose(1, 3, 4, 5, 0, 2).reshape(128, 27, TS2)
        blocks = []
        for kind, r in BLOCKS:
            if kind == "vp":
                blocks.append(vp[lo:hi].reshape(NT, 128, NS)
                              .transpose(1, 0, 2).reshape(128, TS2))
                continue
            x0, nx = ROFF[r], NXS[r]
            if kind == "main":
                blocks.append(qd_c[:, :, x0:x0 + nx].reshape(128, 9 * nx))
                blocks.append(nd_c[:, :, x0:x0 + nx].reshape(128, 9 * nx))
            elif r < 2:
                blocks.append(df12_c[:, :, x0:x0 + nx].reshape(128, 12 * nx))
            else:
                blocks.append(df27_c[:, :, x0:x0 + nx].reshape(128, 27 * nx))
        in2.append({"pk": np.ascontiguousarray(
            np.concatenate(blocks, axis=1), dtype=np.float32)})
    res2 = run_bass_kernel_spmd(p2, in2, list(range(NCORES)), trace=trace, **kw)

    total = np.float64(0.0)
    for c in range(NCORES):
        total += np.asarray(res2.results[c]["wcross"], dtype=np.float64).sum()
    loss = np.float32(total / F)
    return loss, res1, res2, nbr


def run_device(vertices, faces, probabilities, trace=False, **kw):
    loss, res1, res2, _ = _run(vertices, faces, probabilities, trace=trace, **kw)
    return loss, (res1, res2)


def kernel(vertices, faces, probabilities):
    loss, *_ = _run(vertices, faces, probabilities)
    return np.array(loss, dtype=np.float32)
